# revision 6
# baseline (speedup 1.0000x reference)
"""Trainium2 Bass kernel for the sparse-attention scoring module (v2).

Reference computation (S=2048, B=32, H=1024):
    energy[s,b,:]   = enc[s,b,:] @ W^T + bias            # [S,B,H]
    attn[b,s]       = hidden[b,:] . energy[s,b,:]        # [B,S]
    out             = softmax(attn, axis=1)[None]        # [1,B,S]

Algebraic rewrite (as baseline):
    attn[b,s] = (hidden[b,:] @ W) . enc[s,b,:]  (+ const that cancels in
    softmax) -- so the kernel is a streamed batched dot-product over enc.

v2 changes vs the DVE/ACT baseline (140 us):
  1. enc streams as **fp16** (16.8 MB/core instead of 33.5 MB): the HBM
     domain cap (~358 GB/s/core) is the wall, so halving bytes halves the
     stream time.  Logit noise from fp16 is ~0.014 abs on logits of std
     ~38; softmax rows are near-one-hot, so output l2 err is ~3e-3 (sim),
     well under the 2e-2 gate.
  2. The host hands each core enc pre-transposed to [b, h, s] so the
     H-reduction lies on the **partition axis**: the whole dot-product
     becomes TensorE matmuls (lhsT = v-chunk [128,1], rhs = et [128,512],
     PSUM-accumulated over 8 h-chunks into one [4, 2048] tile).  PE does
     ~27 us of work under a ~50 us stream; DVE+ACT are free.
  3. Logits land with **b on partitions**, so the softmax is 5 trivial
     per-partition ops ([4,2048] max / exp+accum / recip / scale) and the
     store needs no transpose.

Sharding: data-parallel over batch, 4 rows per core; W replicated (fp16).
"""

import sys

if "/opt/trn_rl_repo" not in sys.path:
    sys.path.insert(0, "/opt/trn_rl_repo")

import numpy as np

import concourse.bass as bass
import concourse.mybir as mybir
import concourse.tile as tile
from concourse import bacc, bass_utils
from concourse.bass import ts
from concourse.masks import make_identity

S, B, H = 2048, 32, 1024
NCORES = 8
BS = B // NCORES  # 4 batch rows per core
P = 128
KC = H // P  # 8 h-chunks of 128
HALF = 2  # h-chunks per b split into 2 DMA chunks of 4
HCL = KC // HALF
NSB = 4  # s-blocks of 512 per matmul group
SB = S // NSB
F32 = mybir.dt.float32
F16 = mybir.dt.float16
AX = mybir.AxisListType
ALU = mybir.AluOpType
ACT = mybir.ActivationFunctionType

ENC_BUFS = 5


def build_kernel_body(nc, tc, enc, hid_d, w_d, out_d, repeat=1, variant="full"):
    """Emit the per-core program.

    enc is the host-pre-transposed [(b hc p), s] fp16 DRAM tensor.
    repeat > 1 re-runs the streaming loop (timing calibration only).
    variant: "full" | "dmaonly" (skip matmuls) | "computeonly" (skip enc
    DMA; matmuls read a constant tile) for bottleneck isolation.
    """
    import contextlib

    with contextlib.ExitStack() as ctx:
        consts = ctx.enter_context(tc.tile_pool(name="consts", bufs=1))
        wpool = ctx.enter_context(tc.tile_pool(name="wpool", bufs=1))
        epool = ctx.enter_context(tc.tile_pool(name="epool", bufs=ENC_BUFS))
        small = ctx.enter_context(tc.tile_pool(name="small", bufs=1))
        ptp = ctx.enter_context(tc.tile_pool(name="ptp", bufs=2, space="PSUM"))
        plog = ctx.enter_context(tc.tile_pool(name="plog", bufs=1, space="PSUM"))

        identity = consts.tile([P, P], F32)
        make_identity(nc, identity)

        # Preload the exp activation table so the tail doesn't pay ~2.7us.
        warm = small.tile([1, 1], F32)
        nc.gpsimd.memset(warm, 1.0)
        nc.scalar.activation(warm, warm, ACT.Exp)

        # ---- prologue: hidden + W -> vT[p, hc*4 + b] = v[b, hc*128 + p]
        # where v = hidden @ W.  hid/W ride the ACT-issued HWDGE ring so
        # the enc stream owns the sync ring from t=0.
        hid = small.tile([BS, H], F32, tag="hid", name="hid")
        nc.scalar.dma_start(hid, hid_d)

        w_view = w_d.rearrange("(kc kp) h -> kc kp h", kp=P)  # [8, 128, 1024]
        wt = []
        for kc in range(KC):
            wtile = wpool.tile([P, H], F16, tag=f"w{kc}", name=f"w{kc}")
            nc.scalar.dma_start(wtile, w_view[kc])
            wt.append(wtile)

        # hidT[:, 4*kc + b] = hidden[b, kc*128 : (kc+1)*128]  (fp16)
        hidT = small.tile([P, BS * KC], F16, tag="hidT", name="hidT")
        for kc in range(KC):
            pt = ptp.tile([P, BS], F32, tag="pt", name="pt")
            nc.tensor.transpose(pt, hid[:, ts(kc, P)], identity[0:BS, 0:BS])
            nc.scalar.copy(hidT[:, ts(kc, BS)], pt)

        # vT[p, hc*4+b] = sum_k W[k, hc*128+p] * hidden[b, k]
        vT = small.tile([P, KC * BS], F16, tag="vT", name="vT")
        for hc in range(KC):
            pv = ptp.tile([P, BS], F32, tag="pt", name="pv")
            for kc in range(KC):
                nc.tensor.matmul(
                    pv,
                    lhsT=wt[kc][:, ts(hc, P)],
                    rhs=hidT[:, ts(kc, BS)],
                    start=(kc == 0),
                    stop=(kc == KC - 1),
                )
            nc.scalar.copy(vT[:, ts(hc, BS)], pv)

        # ---- main loop: 8 x 2MB fp16 chunks, 16 matmuls each ----
        # enc[(b hc p), s]: chunk (b, half) covers h-chunks half*4..+3.
        enc_view = enc.rearrange(
            "(b half hcl p) s -> b half p hcl s", half=HALF, hcl=HCL, p=P
        )  # [4, 2, 128, 4, 2048]
        # Batch row b accumulates in a [1, 2048] PSUM tile at partition 0;
        # an ACT copy evacuates it and a small SBUF->SBUF DMA moves the row
        # to partition b of logits4 (engines cannot cross partitions).
        logits4 = small.tile([BS, S], F32, tag="logits4", name="logits4")

        cet = None
        if variant == "computeonly":
            cet = epool.tile([P, HCL * S], F16, tag="cet", name="cet")
            nc.gpsimd.memset(cet, 0.001)
        if variant == "dmaonly":
            nc.vector.memset(logits4, 0.0)
        dump = small.tile([P, 1], F32)

        for _rep in range(repeat):
            for b in range(BS):
                plg = plog.tile([1, S], F32, tag="plg", name="plg")
                for half in range(HALF):
                    if variant == "computeonly":
                        et = cet
                    else:
                        et = epool.tile([P, HCL * S], F16, tag="et", name="et")
                        et_v = et.rearrange("p (hcl s) -> p hcl s", hcl=HCL)
                        nc.sync.dma_start(et_v, enc_view[b, half])
                    if variant == "dmaonly":
                        # touch one column so DCE keeps the DMA
                        nc.vector.tensor_scalar_mul(dump, et[:, 0:1], 1.0)
                        continue
                    for hcl in range(HCL):
                        hc = half * HCL + hcl
                        lcol = vT[:, hc * BS + b : hc * BS + b + 1]
                        for sb in range(NSB):
                            nc.tensor.matmul(
                                plg[0:1, ts(sb, SB)],
                                lhsT=lcol,
                                rhs=et[:, hcl * S + sb * SB : hcl * S + (sb + 1) * SB],
                                start=(hc == 0),
                                stop=(hc == KC - 1),
                            )
                if variant != "dmaonly":
                    lgb = small.tile([1, S], F32, tag=f"lg{b}", name=f"lg{b}")
                    nc.scalar.copy(lgb, plg)
                    nc.scalar.dma_start(logits4[b : b + 1, :], lgb)

        # ---- tail: softmax along free dim, b on partitions 0..3 ----
        mx = small.tile([BS, 1], F32)
        nc.vector.tensor_reduce(mx, logits4, axis=AX.X, op=ALU.max)
        negmx = small.tile([BS, 1], F32)
        nc.vector.tensor_scalar_mul(negmx, mx, -1.0)
        probs = small.tile([BS, S], F32)
        sumexp = small.tile([BS, 1], F32)
        nc.scalar.activation(
            probs, logits4, ACT.Exp, bias=negmx, scale=1.0, accum_out=sumexp
        )
        rden = small.tile([BS, 1], F32)
        nc.vector.reciprocal(rden, sumexp)
        outp = small.tile([BS, S], F32)
        nc.vector.tensor_scalar_mul(outp, probs, rden)
        nc.sync.dma_start(out_d, outp)


def build_nc(repeat=1, variant="full"):
    nc = bacc.Bacc(
        "TRN2",
        target_bir_lowering=False,
        debug=False,
        num_devices=NCORES,
    )
    enc = nc.dram_tensor("enc", [BS * KC * P, S], F16, kind="ExternalInput").ap()
    hid_d = nc.dram_tensor("hidden", [BS, H], F32, kind="ExternalInput").ap()
    w_d = nc.dram_tensor("w", [H, H], F16, kind="ExternalInput").ap()
    out_d = nc.dram_tensor("out", [BS, S], F32, kind="ExternalOutput").ap()
    with tile.TileContext(nc) as tc:
        build_kernel_body(
            nc, tc, enc, hid_d, w_d, out_d, repeat=repeat, variant=variant
        )
    nc.compile()
    return nc


def make_in_maps(hidden, encoder_outputs, W):
    hidden = np.asarray(hidden, dtype=np.float32)
    encoder_outputs = np.asarray(encoder_outputs)
    W16 = np.ascontiguousarray(np.asarray(W).astype(np.float16))
    enc16 = encoder_outputs.astype(np.float16)  # [S, B, H]
    in_maps = []
    for c in range(NCORES):
        sl = enc16[:, c * BS : (c + 1) * BS, :]  # [S, 4, H]
        encT = np.empty((BS, H, S), np.float16)
        # blocked transpose for cache locality
        for s0 in range(0, S, 128):
            encT[:, :, s0 : s0 + 128] = sl[s0 : s0 + 128].transpose(1, 2, 0)
        in_maps.append(
            {
                "enc": encT.reshape(BS * H // P * P, S),
                "hidden": np.ascontiguousarray(hidden[c * BS : (c + 1) * BS, :]),
                "w": W16,
            }
        )
    return in_maps


_NC_CACHE = {}


def get_nc():
    if "nc" not in _NC_CACHE:
        _NC_CACHE["nc"] = build_nc()
    return _NC_CACHE["nc"]


def kernel(hidden, encoder_outputs, W, b, **_unused):
    # The linear-layer bias contributes hidden[b].bias to every logit of
    # row b, a per-row constant that cancels in the softmax -> unused.
    nc = get_nc()
    in_maps = make_in_maps(hidden, encoder_outputs, W)
    res = bass_utils.run_bass_kernel_spmd(
        nc, in_maps, core_ids=list(range(NCORES))
    )
    outs = [res.results[c]["out"] for c in range(NCORES)]
    full = np.concatenate(outs, axis=0)  # [32, 2048]
    return full[None, :, :].astype(np.float32, copy=False)


# revision 9
# speedup vs baseline: 1.0186x; 1.0186x over previous
"""Trainium2 Bass kernel for the sparse-attention scoring module (v2).

Reference computation (S=2048, B=32, H=1024):
    energy[s,b,:]   = enc[s,b,:] @ W^T + bias            # [S,B,H]
    attn[b,s]       = hidden[b,:] . energy[s,b,:]        # [B,S]
    out             = softmax(attn, axis=1)[None]        # [1,B,S]

Algebraic rewrite (as baseline):
    attn[b,s] = (hidden[b,:] @ W) . enc[s,b,:]  (+ const that cancels in
    softmax) -- so the kernel is a streamed batched dot-product over enc.

v2 changes vs the DVE/ACT baseline (140 us):
  1. enc streams as **fp16** (16.8 MB/core instead of 33.5 MB): the HBM
     domain cap (~358 GB/s/core) is the wall, so halving bytes halves the
     stream time.  Logit noise from fp16 is ~0.014 abs on logits of std
     ~38; softmax rows are near-one-hot, so output l2 err is ~3e-3 (sim),
     well under the 2e-2 gate.
  2. The host hands each core enc pre-transposed to [b, h, s] so the
     H-reduction lies on the **partition axis**: the whole dot-product
     becomes TensorE matmuls (lhsT = v-chunk [128,1], rhs = et [128,512],
     PSUM-accumulated over 8 h-chunks into one [4, 2048] tile).  PE does
     ~27 us of work under a ~50 us stream; DVE+ACT are free.
  3. Logits land with **b on partitions**, so the softmax is 5 trivial
     per-partition ops ([4,2048] max / exp+accum / recip / scale) and the
     store needs no transpose.

Sharding: data-parallel over batch, 4 rows per core; W replicated (fp16).
"""

import sys

if "/opt/trn_rl_repo" not in sys.path:
    sys.path.insert(0, "/opt/trn_rl_repo")

import numpy as np

import concourse.bass as bass
import concourse.mybir as mybir
import concourse.tile as tile
from concourse import bacc, bass_utils
from concourse.bass import ts
from concourse.masks import make_identity

S, B, H = 2048, 32, 1024
NCORES = 8
BS = B // NCORES  # 4 batch rows per core
P = 128
KC = H // P  # 8 h-chunks of 128
NCH = 4  # DMA chunks per b (1 MB each)
HCL = KC // NCH  # h-chunks per DMA chunk
NSB = 4  # s-blocks of 512 per matmul group
SB = S // NSB
F32 = mybir.dt.float32
F16 = mybir.dt.float16
AX = mybir.AxisListType
ALU = mybir.AluOpType
ACT = mybir.ActivationFunctionType

ENC_BUFS = 8


def build_kernel_body(nc, tc, enc, hid_d, w_d, out_d, repeat=1, variant="full"):
    """Emit the per-core program.

    enc is the host-pre-transposed [(b hc p), s] fp16 DRAM tensor.
    repeat > 1 re-runs the streaming loop (timing calibration only).
    variant: "full" | "dmaonly" (skip matmuls) | "computeonly" (skip enc
    DMA; matmuls read a constant tile) for bottleneck isolation.
    """
    import contextlib

    with contextlib.ExitStack() as ctx:
        consts = ctx.enter_context(tc.tile_pool(name="consts", bufs=1))
        wpool = ctx.enter_context(tc.tile_pool(name="wpool", bufs=1))
        epool = ctx.enter_context(tc.tile_pool(name="epool", bufs=ENC_BUFS))
        small = ctx.enter_context(tc.tile_pool(name="small", bufs=1))
        ptp = ctx.enter_context(tc.tile_pool(name="ptp", bufs=2, space="PSUM"))
        plog = ctx.enter_context(tc.tile_pool(name="plog", bufs=1, space="PSUM"))

        identity = consts.tile([P, P], F32)

        # ---- prologue: hidden + W -> vT[p, hc*4 + b] = v[b, hc*128 + p]
        # where v = hidden @ W.  hid/W ride the ACT-issued HWDGE ring so
        # the enc stream owns the sync ring from t=0.  DMAs issue before
        # anything else occupies the ACT queue.
        hid = small.tile([BS, H], F32, tag="hid", name="hid")
        nc.scalar.dma_start(hid, hid_d)

        w_view = w_d.rearrange("(kc kp) h -> kc kp h", kp=P)  # [8, 128, 1024]
        wt = []
        for kc in range(KC):
            wtile = wpool.tile([P, H], F16, tag=f"w{kc}", name=f"w{kc}")
            nc.scalar.dma_start(wtile, w_view[kc])
            wt.append(wtile)

        make_identity(nc, identity)

        # Preload the exp activation table so the first softmax row doesn't
        # pay the ~2.7us table load on its critical path.
        warm = small.tile([1, 1], F32)
        nc.gpsimd.memset(warm, 1.0)
        nc.scalar.activation(warm, warm, ACT.Exp)

        # hidT[:, 4*kc + b] = hidden[b, kc*128 : (kc+1)*128]  (fp16)
        hidT = small.tile([P, BS * KC], F16, tag="hidT", name="hidT")
        for kc in range(KC):
            pt = ptp.tile([P, BS], F32, tag="pt", name="pt")
            nc.tensor.transpose(pt, hid[:, ts(kc, P)], identity[0:BS, 0:BS])
            nc.scalar.copy(hidT[:, ts(kc, BS)], pt)

        # vT[p, hc*4+b] = sum_k W[k, hc*128+p] * hidden[b, k]
        vT = small.tile([P, KC * BS], F16, tag="vT", name="vT")
        for hc in range(KC):
            pv = ptp.tile([P, BS], F32, tag="pt", name="pv")
            for kc in range(KC):
                nc.tensor.matmul(
                    pv,
                    lhsT=wt[kc][:, ts(hc, P)],
                    rhs=hidT[:, ts(kc, BS)],
                    start=(kc == 0),
                    stop=(kc == KC - 1),
                )
            nc.scalar.copy(vT[:, ts(hc, BS)], pv)

        # ---- main loop: 8 x 2MB fp16 chunks, 16 matmuls each ----
        # enc[(b hc p), s]: chunk (b, half) covers h-chunks half*4..+3.
        enc_view = enc.rearrange(
            "(b ch hcl p) s -> b ch p hcl s", ch=NCH, hcl=HCL, p=P
        )  # [4, NCH, 128, HCL, 2048]
        # Batch row b accumulates in a [1, 2048] PSUM tile at partition 0.
        # Its softmax runs immediately after evacuation, overlapped with the
        # next rows' streaming: only row 3's epilogue is exposed at the end.
        cet = None
        if variant == "computeonly":
            cet = epool.tile([P, HCL * S], F16, tag="cet", name="cet")
            nc.gpsimd.memset(cet, 0.001)
        dump = small.tile([P, 1], F32)

        for _rep in range(repeat):
            for b in range(BS):
                plg = plog.tile([1, S], F32, tag="plg", name="plg")
                for ch in range(NCH):
                    if variant == "computeonly":
                        et = cet
                    else:
                        et = epool.tile([P, HCL * S], F16, tag="et", name="et")
                        et_v = et.rearrange("p (hcl s) -> p hcl s", hcl=HCL)
                        nc.sync.dma_start(et_v, enc_view[b, ch])
                    if variant == "dmaonly":
                        # touch one column so DCE keeps the DMA
                        nc.vector.tensor_scalar_mul(dump, et[:, 0:1], 1.0)
                        continue
                    for hcl in range(HCL):
                        hc = ch * HCL + hcl
                        lcol = vT[:, hc * BS + b : hc * BS + b + 1]
                        for sb in range(NSB):
                            nc.tensor.matmul(
                                plg[0:1, ts(sb, SB)],
                                lhsT=lcol,
                                rhs=et[:, hcl * S + sb * SB : hcl * S + (sb + 1) * SB],
                                start=(hc == 0),
                                stop=(hc == KC - 1),
                            )
                if variant == "dmaonly":
                    continue
                # ---- row epilogue: evacuate + softmax + store ----
                lgb = small.tile([1, S], F32, tag=f"lg{b}", name=f"lg{b}")
                nc.scalar.copy(lgb, plg)
                mx = small.tile([1, 1], F32, tag=f"mx{b}")
                nc.vector.tensor_reduce(mx, lgb, axis=AX.X, op=ALU.max)
                negmx = small.tile([1, 1], F32, tag=f"nm{b}")
                nc.vector.tensor_scalar_mul(negmx, mx, -1.0)
                probs = small.tile([1, S], F32, tag=f"pr{b}")
                sumexp = small.tile([1, 1], F32, tag=f"se{b}")
                nc.scalar.activation(
                    probs, lgb, ACT.Exp, bias=negmx, scale=1.0, accum_out=sumexp
                )
                rden = small.tile([1, 1], F32, tag=f"rd{b}")
                nc.vector.reciprocal(rden, sumexp)
                outp = small.tile([1, S], F32, tag=f"ou{b}")
                nc.vector.tensor_scalar_mul(outp, probs, rden)
                # store on the (idle) ACT ring so it never queues behind
                # upcoming enc chunks on the sync ring
                nc.scalar.dma_start(out_d[b : b + 1, :], outp)


def build_nc(repeat=1, variant="full"):
    nc = bacc.Bacc(
        "TRN2",
        target_bir_lowering=False,
        debug=False,
        num_devices=NCORES,
    )
    enc = nc.dram_tensor("enc", [BS * KC * P, S], F16, kind="ExternalInput").ap()
    hid_d = nc.dram_tensor("hidden", [BS, H], F32, kind="ExternalInput").ap()
    w_d = nc.dram_tensor("w", [H, H], F16, kind="ExternalInput").ap()
    out_d = nc.dram_tensor("out", [BS, S], F32, kind="ExternalOutput").ap()
    with tile.TileContext(nc) as tc:
        build_kernel_body(
            nc, tc, enc, hid_d, w_d, out_d, repeat=repeat, variant=variant
        )
    nc.compile()
    return nc


def make_in_maps(hidden, encoder_outputs, W):
    hidden = np.asarray(hidden, dtype=np.float32)
    encoder_outputs = np.asarray(encoder_outputs)
    W16 = np.ascontiguousarray(np.asarray(W).astype(np.float16))
    enc16 = encoder_outputs.astype(np.float16)  # [S, B, H]
    in_maps = []
    for c in range(NCORES):
        sl = enc16[:, c * BS : (c + 1) * BS, :]  # [S, 4, H]
        encT = np.empty((BS, H, S), np.float16)
        # blocked transpose for cache locality
        for s0 in range(0, S, 128):
            encT[:, :, s0 : s0 + 128] = sl[s0 : s0 + 128].transpose(1, 2, 0)
        in_maps.append(
            {
                "enc": encT.reshape(BS * H // P * P, S),
                "hidden": np.ascontiguousarray(hidden[c * BS : (c + 1) * BS, :]),
                "w": W16,
            }
        )
    return in_maps


_NC_CACHE = {}


def get_nc():
    if "nc" not in _NC_CACHE:
        _NC_CACHE["nc"] = build_nc()
    return _NC_CACHE["nc"]


def kernel(hidden, encoder_outputs, W, b, **_unused):
    # The linear-layer bias contributes hidden[b].bias to every logit of
    # row b, a per-row constant that cancels in the softmax -> unused.
    nc = get_nc()
    in_maps = make_in_maps(hidden, encoder_outputs, W)
    res = bass_utils.run_bass_kernel_spmd(
        nc, in_maps, core_ids=list(range(NCORES))
    )
    outs = [res.results[c]["out"] for c in range(NCORES)]
    full = np.concatenate(outs, axis=0)  # [32, 2048]
    return full[None, :, :].astype(np.float32, copy=False)
